# revision 22
# baseline (speedup 1.0000x reference)
"""Min-Euclidean-distance retrieval kernel for Trainium2 (8 NeuronCores).

Reference computation:
    x: [1, 2048, 512], y: [1, 65536, 512] (fp32)
    sq[p, r] = ||x_p||^2 + ||y_r||^2 - 2 <x_p, y_r>
    out = min over (p, r) of sqrt(max(sq, 0))

Sharding: candidate pool (R) split across 8 cores, 8192 candidates each.
Host pre-arranges both GEMM operands partition-major in fp8 so each DMA
moves contiguous per-partition runs and the contraction dim lands on SBUF
partitions with no on-chip transposes.

Per core the hot loop is 64 candidate tiles of [128 cand x 2048 queries],
8 fp8 DoubleRow matmuls each (216ns issue floor -> 1.728us/tile, the PE
roofline).  The 2048 query columns split three ways so no sidecar engine
exceeds the PE tile period:
  cols    0-1023  ScalarE ACTIVATE h=-2*pa+(y2-512) (with 1024:1536,
                  1.54us) then VectorE fp16 tensor_tensor running min
                  into acc_a (0.69us)
  cols 1024-1535  same ACTIVATE output, shipped raw to DRAM on the sync
                  HWDGE ring (host takes the min over tiles)
  cols 1536-2047  VectorE scalar_tensor_tensor straight off PSUM:
                  acc_m = max(acc_m, pb - y2/2)  (0.75us; bias + running
                  reduce fused, nothing shipped)
VectorE ~1.49us/tile, ScalarE ~1.59us/tile, both under the PE's 1.728us.

Queue discipline:
  - scalar (Activation) queue: head x/y DMA issues only BEFORE the first
    ACTIVATE; nothing mid-stream (ACTIVATE shares the queue).
  - gpsimd: memsets only (SWDGE transfers measured ~65 GB/s and its
    block-exit DRAIN blocked 11us when it carried y tiles).
  - h tiles: 16 single-tile buffers -- deep enough that ACTIVATE's WAR
    wait on the hbd ship of t-16 never binds, with no pair-packing WAW
    semaphores on the scalar queue.
  - x arrives as 8 consumption-ordered 128KB chunk DMAs alternating
    sync/scalar (first real MM ~8.7us); y tiles 0-1 lead on sync, the
    rest stream in 4-tile groups from inside the loop on sync,
    interleaved with the hbd ships.

Garbage warm-up matmuls (memset FIRST on gpsimd, before everything)
bridge the PE clock ramp across the input-DMA window.  The final tile's
ACTIVATE is split in two so the closing tensor_tensor min and the outa
ship start ~0.4us earlier.

The per-query ||x_p||^2 term commutes with the min over candidates and
is added on the host, with the final min across lanes/cores/tiles and
the (monotone) sqrt. fp8 GEMM + fp16 epilogue measure ~1.6e-3 relative
error on the final distance, well inside the 2e-2 tolerance.
"""

import os
import sys

# Recover automatically if a previous process left the NeuronCores wedged.
os.environ.setdefault("NEURON_RT_RESET_CORES", "1")

for _p in ("/opt/trn_rl_repo", "/root/.axon_site/_ro/trn_rl_repo"):
    if _p not in sys.path:
        sys.path.append(_p)

import ml_dtypes
import numpy as np

import concourse.bass as bass
import concourse.mybir as mybir
import concourse.tile as tile
from concourse import bacc, bass_utils

P = 2048          # queries
R = 65536         # candidates (full)
D = 512           # feature dim
NCORES = 8
R_LOC = R // NCORES      # 8192 candidates per core
P_CHUNKS = P // 512      # 4 query chunks (one PSUM bank each)
R_TILES = R_LOC // 128   # 64 candidate tiles
K_TILES = D // 128       # 4 contraction tiles (2 DoubleRow passes)
PA = 1024                # query cols on the ScalarE+VectorE-min path
PS = 512                 # query cols shipped raw to the host
PM = 512                 # query cols on the fused VectorE max path
PAS = PA + PS

F32 = mybir.dt.float32
MM_DT = mybir.dt.float8e4
MM_NP = ml_dtypes.float8_e4m3
ACC_DT, ACC_NP = mybir.dt.float16, np.float16
# The a-path epilogue runs in fp16. A constant shift keeps the values that
# matter (near the global min, sq ~ 650 => h ~ 150) small; fp16 quantum
# there is ~0.125, negligible next to the fp8 GEMM noise.
Y2_SHIFT = np.float32(512.0)
N_WARM = 4        # garbage matmuls bridging the input-DMA window


def _build_module() -> bass.Bass:
    nc = bacc.Bacc("TRN2", target_bir_lowering=False, debug=False)

    # Host-prepared layouts (partition-major, contiguous per partition):
    #   xt[q, c, k, j]   = x[c*512 + j, k*128 + q]
    #   yt[q, t, k, s]   = y[t*128 + s, k*128 + q]  (t-major: one candidate
    #                      tile = one contiguous 512B-per-partition slice)
    #   y2bh[lane, 0, t] = ||y_r||^2 - Y2_SHIFT for r = t*128 + lane
    #   y2bh[lane, 1, t] = ||y_r||^2 / 2
    xt = nc.dram_tensor("xt", [128, P_CHUNKS, K_TILES, 512], MM_DT,
                        kind="ExternalInput")
    yt = nc.dram_tensor("yt", [128, R_TILES, K_TILES, 128], MM_DT,
                        kind="ExternalInput")
    y2bh = nc.dram_tensor("y2bh", [128, 2, R_TILES], F32, kind="ExternalInput")
    # outa[lane, j<PA]: min over tiles t of (y2[t*128+lane] - 512 - 2 G[., j])
    outa = nc.dram_tensor("outa", [128, PA], ACC_DT, kind="ExternalOutput")
    # outm[lane, j<PM]: max over tiles t of (G[., PA+PS+j] - y2[t*128+lane]/2)
    outm = nc.dram_tensor("outm", [128, PM], ACC_DT, kind="ExternalOutput")
    # hbd[lane, t, j] = y2[t*128+lane] - 512 - 2 G[t*128+lane, PA+j] (no min)
    hbd = nc.dram_tensor("hbd", [128, R_TILES, PS], ACC_DT,
                         kind="ExternalOutput")

    with tile.TileContext(nc) as tc:
        with (
            tc.tile_pool(name="big", bufs=1) as big,
            tc.tile_pool(name="hpool", bufs=16) as hpool,
            tc.tile_pool(name="psa", bufs=2, space="PSUM") as psa,
            tc.tile_pool(name="psb", bufs=2, space="PSUM") as psb,
        ):
            garb = big.tile([128, 2, 512], MM_DT)
            xt_sb = big.tile([128, P_CHUNKS, K_TILES, 512], MM_DT)
            yt_sb = big.tile([128, R_TILES, K_TILES, 128], MM_DT)
            y2bh_sb = big.tile([128, 2, R_TILES], F32)
            acc_a = big.tile([128, PA], ACC_DT)
            acc_m = big.tile([128, PM], ACC_DT)

            # GpSimd zeroes the warm-up operand FIRST (so the warm-ups are
            # schedulable immediately), then seeds the accumulators.
            nc.gpsimd.memset(garb[:], 0)
            nc.gpsimd.memset(acc_a[:], float("inf"))
            nc.gpsimd.memset(acc_m[:], float("-inf"))

            # Leading-edge DMAs: y0-1 lead (the first LDWEIGHTS gate), then
            # the 8 x chunk-pairs in exact MM-consumption order alternating
            # scalar/sync, then the next y tiles.  All of scalar's issues
            # are pre-ACTIVATE.
            nc.sync.dma_start(yt_sb[:, 0:2], yt.ap()[:, 0:2])
            nc.scalar.dma_start(xt_sb[:, 0, 0:2], xt.ap()[:, 0, 0:2])
            nc.sync.dma_start(xt_sb[:, 1, 0:2], xt.ap()[:, 1, 0:2])
            nc.scalar.dma_start(xt_sb[:, 2, 0:2], xt.ap()[:, 2, 0:2])
            nc.sync.dma_start(xt_sb[:, 3, 0:2], xt.ap()[:, 3, 0:2])
            nc.scalar.dma_start(y2bh_sb[:], y2bh.ap())
            nc.scalar.dma_start(xt_sb[:, 0, 2:4], xt.ap()[:, 0, 2:4])
            nc.sync.dma_start(xt_sb[:, 1, 2:4], xt.ap()[:, 1, 2:4])
            nc.scalar.dma_start(xt_sb[:, 2, 2:4], xt.ap()[:, 2, 2:4])
            nc.sync.dma_start(xt_sb[:, 3, 2:4], xt.ap()[:, 3, 2:4])
            nc.sync.dma_start(yt_sb[:, 2:4], yt.ap()[:, 2:4])
            nc.scalar.dma_start(yt_sb[:, 4:8], yt.ap()[:, 4:8])

            # Warm-up matmuls on the zeroed garbage tile: the PE would
            # otherwise sit idle waiting for x/y and spend the first real
            # tiles at the ramp-up clock.
            pwarm = psb.tile([128, PM], F32, name="pb")
            for _ in range(N_WARM):
                nc.tensor.matmul(
                    pwarm[:, 0:256],
                    lhsT=garb[:, :, 0:128],
                    rhs=garb[:, :, 0:256],
                    start=True,
                    stop=True,
                    perf_mode=mybir.MatmulPerfMode.DoubleRow,
                )

            prev_h = None
            for t in range(R_TILES):
                if t % 8 == 2 and t < 51:
                    # y tiles for group g..g+7 (issued 6+ tiles ahead).
                    # Issued at t=2 mod 8: the first issue's semaphore
                    # recycling then lands AFTER the first fused-max on the
                    # vector queue -- at t=0 its guard blocked STT(0) on an
                    # unrelated head x-chunk transfer for ~2us.  8-tile
                    # groups halve the issue count and the exit teardown.
                    g = 6 + t
                    nc.sync.dma_start(yt_sb[:, g : g + 8], yt.ap()[:, g : g + 8])
                pa = psa.tile([128, PAS], F32, name="pa")
                pb = psb.tile([128, PM], F32, name="pb")
                # kk outer keeps the stationary operand loaded across
                # chunks; c3 (the fused-max path) last so pa completes at
                # MM#7 and ScalarE starts one MM early.
                # kk0 leads with c3 so each tile's first MM writes pb
                # (gated by the fast fused-max) instead of pa (gated by the
                # 1.79us ACTIVATE chain), buying ScalarE one MM period.
                for kk in range(K_TILES // 2):
                    for c in ((3, 0, 1, 2) if kk == 0 else (0, 1, 2, 3)):
                        dst = (pa[:, c * 512 : (c + 1) * 512]
                               if c < 3 else pb[:])
                        nc.tensor.matmul(
                            dst,
                            lhsT=yt_sb[:, t, 2 * kk : 2 * kk + 2],
                            rhs=xt_sb[:, c, 2 * kk : 2 * kk + 2],
                            start=(kk == 0),
                            stop=(kk == K_TILES // 2 - 1),
                            perf_mode=mybir.MatmulPerfMode.DoubleRow,
                        )
                h = hpool.tile([128, PAS], ACC_DT, name="h")
                if t == R_TILES - 1:
                    # Split so the closing min (and outa) starts early.
                    nc.scalar.activation(
                        out=h[:, 0:PA], in_=pa[:, 0:PA],
                        func=mybir.ActivationFunctionType.Identity,
                        bias=y2bh_sb[:, 0, t : t + 1], scale=-2.0,
                    )
                    nc.scalar.activation(
                        out=h[:, PA:PAS], in_=pa[:, PA:PAS],
                        func=mybir.ActivationFunctionType.Identity,
                        bias=y2bh_sb[:, 0, t : t + 1], scale=-2.0,
                    )
                else:
                    nc.scalar.activation(
                        out=h[:],
                        in_=pa[:],
                        func=mybir.ActivationFunctionType.Identity,
                        bias=y2bh_sb[:, 0, t : t + 1],
                        scale=-2.0,
                    )
                # The a-path min runs one tile late so the fused-max (which
                # frees the psb buffer) never queues behind it.
                if prev_h is not None:
                    nc.vector.tensor_tensor(
                        out=acc_a[:], in0=acc_a[:], in1=prev_h[:, 0:PA],
                        op=mybir.AluOpType.min,
                    )
                nc.vector.scalar_tensor_tensor(
                    out=acc_m[:], in0=pb[:], scalar=y2bh_sb[:, 1, t : t + 1],
                    in1=acc_m[:],
                    op0=mybir.AluOpType.subtract, op1=mybir.AluOpType.max,
                )
                if t == R_TILES - 1:
                    nc.sync.dma_start(outm.ap(), acc_m[:])
                nc.sync.dma_start(hbd.ap()[:, t], h[:, PA:PAS])
                prev_h = h
            nc.vector.tensor_tensor(
                out=acc_a[:], in0=acc_a[:], in1=prev_h[:, 0:PA],
                op=mybir.AluOpType.min,
            )
            nc.scalar.dma_start(outa.ap(), acc_a[:])
    nc.compile()
    return nc


_module_cache: bass.Bass | None = None


def _get_module() -> bass.Bass:
    global _module_cache
    if _module_cache is None:
        _module_cache = _build_module()
    return _module_cache


def _prepare_inputs(x: np.ndarray, y: np.ndarray):
    """Host-side sharding/layout prep. Returns per-core input maps."""
    # xt[q, c, k, j] = x[c*512 + j, k*128 + q]
    xt4 = x.T.reshape(K_TILES, 128, P_CHUNKS, 512)
    xt = np.ascontiguousarray(xt4.transpose(1, 2, 0, 3).astype(MM_NP))
    in_maps = []
    for cc in range(NCORES):
        yc = y[cc * R_LOC : (cc + 1) * R_LOC]
        # yt[q, t, k, s] = yc[t*128 + s, k*128 + q]
        a = yc.reshape(R_TILES, 128, K_TILES, 128)
        yct = np.ascontiguousarray(a.transpose(3, 0, 2, 1).astype(MM_NP))
        y2 = np.einsum("rd,rd->r", yc, yc, dtype=np.float32)
        y2bh = np.stack([
            (y2 - Y2_SHIFT).reshape(R_TILES, 128).T,
            (y2 * np.float32(0.5)).reshape(R_TILES, 128).T,
        ], axis=1)
        in_maps.append({"xt": xt, "yt": yct,
                        "y2bh": np.ascontiguousarray(y2bh)})
    return in_maps


def _postprocess(x: np.ndarray, outas: np.ndarray, outms: np.ndarray,
                 hbds: np.ndarray) -> np.ndarray:
    """outas: [NCORES,128,PA]; outms: [NCORES,128,PM]; hbds: [NCORES,128,T,PS]."""
    x2 = np.einsum("pd,pd->p", x, x, dtype=np.float32)
    # cols [0, PA): h accumulated as y2 - shift - 2G, min'd on-chip
    ma = outas.astype(np.float32).min(axis=(0, 1)) + Y2_SHIFT
    # cols [PA, PA+PS): raw h tiles, min on host
    mb = hbds.astype(np.float32).min(axis=(0, 1, 2)) + Y2_SHIFT
    # cols [PA+PS, P): acc of max(G - y2/2); sq = x2 - 2*max
    mm = np.float32(-2.0) * outms.astype(np.float32).max(axis=(0, 1))
    m = np.concatenate([ma, mb, mm])
    sq_min = np.float32((x2 + m).min())
    return np.sqrt(np.maximum(sq_min, np.float32(0.0)), dtype=np.float32)


def kernel(
    predicted_transaction_company: np.ndarray,
    future_transaction_companies_inc_current_data: np.ndarray,
) -> np.ndarray:
    x = np.asarray(predicted_transaction_company, dtype=np.float32)[0]
    y = np.asarray(future_transaction_companies_inc_current_data, dtype=np.float32)[0]

    nc = _get_module()
    in_maps = _prepare_inputs(x, y)
    res = bass_utils.run_bass_kernel_spmd(nc, in_maps, core_ids=list(range(NCORES)))
    outas = np.stack([r["outa"] for r in res.results])
    outms = np.stack([r["outm"] for r in res.results])
    hbds = np.stack([r["hbd"] for r in res.results])
    return _postprocess(x, outas, outms, hbds)


# revision 26
# speedup vs baseline: 1.0124x; 1.0124x over previous
"""Min-Euclidean-distance retrieval kernel for Trainium2 (8 NeuronCores).

Reference computation:
    x: [1, 2048, 512], y: [1, 65536, 512] (fp32)
    sq[p, r] = ||x_p||^2 + ||y_r||^2 - 2 <x_p, y_r>
    out = min over (p, r) of sqrt(max(sq, 0))

Sharding: candidate pool (R) split across 8 cores, 8192 candidates each.
Host pre-arranges both GEMM operands partition-major in fp8 so each DMA
moves contiguous per-partition runs and the contraction dim lands on SBUF
partitions with no on-chip transposes.

Per core the hot loop is 64 candidate tiles of [128 cand x 2048 queries],
8 fp8 DoubleRow matmuls each (216ns issue floor -> 1.728us/tile, the PE
roofline).  The 2048 query columns split three ways so no sidecar engine
exceeds the PE tile period:
  cols    0-1023  ScalarE ACTIVATE h=-2*pa+(y2-512) (with 1024:1536,
                  1.54us) then VectorE fp16 tensor_tensor running min
                  into acc_a (0.69us)
  cols 1024-1535  same ACTIVATE output, shipped raw to DRAM on the sync
                  HWDGE ring (host takes the min over tiles)
  cols 1536-2047  VectorE scalar_tensor_tensor straight off PSUM:
                  acc_m = max(acc_m, pb - y2/2)  (0.75us; bias + running
                  reduce fused, nothing shipped)
VectorE ~1.49us/tile, ScalarE ~1.59us/tile, both under the PE's 1.728us.

Queue discipline:
  - scalar (Activation) queue: head x/y DMA issues only BEFORE the first
    ACTIVATE; nothing mid-stream (ACTIVATE shares the queue).
  - gpsimd: memsets only (SWDGE transfers measured ~65 GB/s and its
    block-exit DRAIN blocked 11us when it carried y tiles).
  - h tiles: 16 single-tile buffers -- deep enough that ACTIVATE's WAR
    wait on the hbd ship of t-16 never binds, with no pair-packing WAW
    semaphores on the scalar queue.
  - x arrives as 8 consumption-ordered 128KB chunk DMAs alternating
    sync/scalar (first real MM ~8.7us); y tiles 0-1 lead on sync, the
    rest stream in 4-tile groups from inside the loop on sync,
    interleaved with the hbd ships.

Garbage warm-up matmuls (memset FIRST on gpsimd, before everything)
bridge the PE clock ramp across the input-DMA window.  The final tile's
ACTIVATE is split in two so the closing tensor_tensor min and the outa
ship start ~0.4us earlier.

The per-query ||x_p||^2 term commutes with the min over candidates and
is added on the host, with the final min across lanes/cores/tiles and
the (monotone) sqrt. fp8 GEMM + fp16 epilogue measure ~1.6e-3 relative
error on the final distance, well inside the 2e-2 tolerance.
"""

import os
import sys

# Recover automatically if a previous process left the NeuronCores wedged.
os.environ.setdefault("NEURON_RT_RESET_CORES", "1")

for _p in ("/opt/trn_rl_repo", "/root/.axon_site/_ro/trn_rl_repo"):
    if _p not in sys.path:
        sys.path.append(_p)

import ml_dtypes
import numpy as np

import concourse.bass as bass
import concourse.mybir as mybir
import concourse.tile as tile
from concourse import bacc, bass_utils

P = 2048          # queries
R = 65536         # candidates (full)
D = 512           # feature dim
NCORES = 8
R_LOC = R // NCORES      # 8192 candidates per core
P_CHUNKS = P // 512      # 4 query chunks (one PSUM bank each)
R_TILES = R_LOC // 128   # 64 candidate tiles
K_TILES = D // 128       # 4 contraction tiles (2 DoubleRow passes)
PA = 1024                # query cols on the ScalarE+VectorE-min path
PS = 512                 # query cols shipped raw to the host
PM = 512                 # query cols on the fused VectorE max path
PAS = PA + PS

F32 = mybir.dt.float32
MM_DT = mybir.dt.float8e4
MM_NP = ml_dtypes.float8_e4m3
ACC_DT, ACC_NP = mybir.dt.float16, np.float16
# The a-path epilogue runs in fp16. A constant shift keeps the values that
# matter (near the global min, sq ~ 650 => h ~ 150) small; fp16 quantum
# there is ~0.125, negligible next to the fp8 GEMM noise.
Y2_SHIFT = np.float32(512.0)
# Garbage matmuls bridging the input-DMA window: enough to keep the PE
# busy from ~8.2us until the first x chunk lands (~10.5us) -- a gap there
# resets the clock-ramp timer and the first ~10 real matmuls then run at
# the mid pstate (426ns instead of 216ns, ~1.8us lost).
N_WARM = 10


def _build_module() -> bass.Bass:
    nc = bacc.Bacc("TRN2", target_bir_lowering=False, debug=False)

    # Host-prepared layouts (partition-major, contiguous per partition):
    #   xt[q, c, k, j]   = x[c*512 + j, k*128 + q]
    #   yt[q, t, k, s]   = y[t*128 + s, k*128 + q]  (t-major: one candidate
    #                      tile = one contiguous 512B-per-partition slice)
    #   y2bh[lane, 0, t] = ||y_r||^2 - Y2_SHIFT for r = t*128 + lane
    #   y2bh[lane, 1, t] = ||y_r||^2 / 2
    xt = nc.dram_tensor("xt", [128, P_CHUNKS, K_TILES, 512], MM_DT,
                        kind="ExternalInput")
    yt = nc.dram_tensor("yt", [128, R_TILES, K_TILES, 128], MM_DT,
                        kind="ExternalInput")
    y2bh = nc.dram_tensor("y2bh", [128, 2, R_TILES], F32, kind="ExternalInput")
    # outa[lane, j<PA]: min over tiles t of (y2[t*128+lane] - 512 - 2 G[., j])
    outa = nc.dram_tensor("outa", [128, PA], ACC_DT, kind="ExternalOutput")
    # outm[lane, j<PM]: max over tiles t of (G[., PA+PS+j] - y2[t*128+lane]/2)
    outm = nc.dram_tensor("outm", [128, PM], ACC_DT, kind="ExternalOutput")
    # hbd[lane, t, j] = y2[t*128+lane] - 512 - 2 G[t*128+lane, PA+j] (no min)
    hbd = nc.dram_tensor("hbd", [128, R_TILES, PS], ACC_DT,
                         kind="ExternalOutput")

    with tile.TileContext(nc) as tc:
        with (
            tc.tile_pool(name="big", bufs=1) as big,
            tc.tile_pool(name="hpool", bufs=16) as hpool,
            tc.tile_pool(name="psa", bufs=2, space="PSUM") as psa,
            tc.tile_pool(name="psb", bufs=2, space="PSUM") as psb,
        ):
            garb = big.tile([128, 2, 512], MM_DT)
            xt_sb = big.tile([128, P_CHUNKS, K_TILES, 512], MM_DT)
            yt_sb = big.tile([128, R_TILES, K_TILES, 128], MM_DT)
            y2bh_sb = big.tile([128, 2, R_TILES], F32)
            acc_a = big.tile([128, PA], ACC_DT)
            acc_m = big.tile([128, PM], ACC_DT)

            # GpSimd zeroes the warm-up operand FIRST (so the warm-ups are
            # schedulable immediately), then seeds the accumulators.
            nc.gpsimd.memset(garb[:], 0)
            nc.gpsimd.memset(acc_a[:], float("inf"))
            nc.gpsimd.memset(acc_m[:], float("-inf"))

            # Leading-edge DMAs: y0-1 lead (the first LDWEIGHTS gate), then
            # the 8 x chunk-pairs in exact MM-consumption order alternating
            # scalar/sync, then the next y tiles.  All of scalar's issues
            # are pre-ACTIVATE.
            nc.sync.dma_start(yt_sb[:, 0:2], yt.ap()[:, 0:2])
            nc.scalar.dma_start(xt_sb[:, 0, 0:2], xt.ap()[:, 0, 0:2])
            nc.sync.dma_start(xt_sb[:, 1, 0:2], xt.ap()[:, 1, 0:2])
            nc.scalar.dma_start(xt_sb[:, 2, 0:2], xt.ap()[:, 2, 0:2])
            nc.sync.dma_start(xt_sb[:, 3, 0:2], xt.ap()[:, 3, 0:2])
            nc.scalar.dma_start(y2bh_sb[:], y2bh.ap())
            nc.scalar.dma_start(xt_sb[:, 0, 2:4], xt.ap()[:, 0, 2:4])
            nc.sync.dma_start(xt_sb[:, 1, 2:4], xt.ap()[:, 1, 2:4])
            nc.scalar.dma_start(xt_sb[:, 2, 2:4], xt.ap()[:, 2, 2:4])
            nc.sync.dma_start(xt_sb[:, 3, 2:4], xt.ap()[:, 3, 2:4])
            nc.sync.dma_start(yt_sb[:, 2:4], yt.ap()[:, 2:4])
            nc.scalar.dma_start(yt_sb[:, 4:8], yt.ap()[:, 4:8])

            # Warm-up matmuls on the zeroed garbage tile: the PE would
            # otherwise sit idle waiting for x/y and spend the first real
            # tiles at the ramp-up clock.
            pwarm = psb.tile([128, PM], F32, name="pb")
            for _ in range(N_WARM):
                nc.tensor.matmul(
                    pwarm[:, 0:256],
                    lhsT=garb[:, :, 0:128],
                    rhs=garb[:, :, 0:256],
                    start=True,
                    stop=True,
                    perf_mode=mybir.MatmulPerfMode.DoubleRow,
                )

            prev_h = None
            for t in range(R_TILES):
                if t % 4 == 2 and t < 58:
                    # y tiles for group g..g+3 (issued 6+ tiles ahead).
                    # Issued at t=2 mod 4 so the first issue's semaphore
                    # recycling lands AFTER the first fused-max on the
                    # vector queue -- at t=0 its guard blocked STT(0) on an
                    # unrelated head x-chunk transfer for ~2us.
                    g = 6 + t
                    nc.sync.dma_start(yt_sb[:, g : g + 4], yt.ap()[:, g : g + 4])
                pa = psa.tile([128, PAS], F32, name="pa")
                pb = psb.tile([128, PM], F32, name="pb")
                # kk outer keeps the stationary operand loaded across
                # chunks; c3 (the fused-max path) last so pa completes at
                # MM#7 and ScalarE starts one MM early.
                for kk in range(K_TILES // 2):
                    for c in range(P_CHUNKS):
                        dst = (pa[:, c * 512 : (c + 1) * 512]
                               if c < 3 else pb[:])
                        nc.tensor.matmul(
                            dst,
                            lhsT=yt_sb[:, t, 2 * kk : 2 * kk + 2],
                            rhs=xt_sb[:, c, 2 * kk : 2 * kk + 2],
                            start=(kk == 0),
                            stop=(kk == K_TILES // 2 - 1),
                            perf_mode=mybir.MatmulPerfMode.DoubleRow,
                        )
                h = hpool.tile([128, PAS], ACC_DT, name="h")
                if t == R_TILES - 1:
                    # Split so the closing min (and outa) starts early.
                    nc.scalar.activation(
                        out=h[:, 0:PA], in_=pa[:, 0:PA],
                        func=mybir.ActivationFunctionType.Identity,
                        bias=y2bh_sb[:, 0, t : t + 1], scale=-2.0,
                    )
                    nc.scalar.activation(
                        out=h[:, PA:PAS], in_=pa[:, PA:PAS],
                        func=mybir.ActivationFunctionType.Identity,
                        bias=y2bh_sb[:, 0, t : t + 1], scale=-2.0,
                    )
                else:
                    nc.scalar.activation(
                        out=h[:],
                        in_=pa[:],
                        func=mybir.ActivationFunctionType.Identity,
                        bias=y2bh_sb[:, 0, t : t + 1],
                        scale=-2.0,
                    )
                # The a-path min runs one tile late so the fused-max (which
                # frees the psb buffer) never queues behind it.
                if prev_h is not None:
                    nc.vector.tensor_tensor(
                        out=acc_a[:], in0=acc_a[:], in1=prev_h[:, 0:PA],
                        op=mybir.AluOpType.min,
                    )
                nc.vector.scalar_tensor_tensor(
                    out=acc_m[:], in0=pb[:], scalar=y2bh_sb[:, 1, t : t + 1],
                    in1=acc_m[:],
                    op0=mybir.AluOpType.subtract, op1=mybir.AluOpType.max,
                )
                if t == R_TILES - 1:
                    nc.sync.dma_start(outm.ap(), acc_m[:])
                nc.sync.dma_start(hbd.ap()[:, t], h[:, PA:PAS])
                prev_h = h
            nc.vector.tensor_tensor(
                out=acc_a[:], in0=acc_a[:], in1=prev_h[:, 0:PA],
                op=mybir.AluOpType.min,
            )
            nc.scalar.dma_start(outa.ap(), acc_a[:])
    nc.compile()
    return nc


_module_cache: bass.Bass | None = None


def _get_module() -> bass.Bass:
    global _module_cache
    if _module_cache is None:
        _module_cache = _build_module()
    return _module_cache


def _prepare_inputs(x: np.ndarray, y: np.ndarray):
    """Host-side sharding/layout prep. Returns per-core input maps."""
    # xt[q, c, k, j] = x[c*512 + j, k*128 + q]
    xt4 = x.T.reshape(K_TILES, 128, P_CHUNKS, 512)
    xt = np.ascontiguousarray(xt4.transpose(1, 2, 0, 3).astype(MM_NP))
    in_maps = []
    for cc in range(NCORES):
        yc = y[cc * R_LOC : (cc + 1) * R_LOC]
        # yt[q, t, k, s] = yc[t*128 + s, k*128 + q]
        a = yc.reshape(R_TILES, 128, K_TILES, 128)
        yct = np.ascontiguousarray(a.transpose(3, 0, 2, 1).astype(MM_NP))
        y2 = np.einsum("rd,rd->r", yc, yc, dtype=np.float32)
        y2bh = np.stack([
            (y2 - Y2_SHIFT).reshape(R_TILES, 128).T,
            (y2 * np.float32(0.5)).reshape(R_TILES, 128).T,
        ], axis=1)
        in_maps.append({"xt": xt, "yt": yct,
                        "y2bh": np.ascontiguousarray(y2bh)})
    return in_maps


def _postprocess(x: np.ndarray, outas: np.ndarray, outms: np.ndarray,
                 hbds: np.ndarray) -> np.ndarray:
    """outas: [NCORES,128,PA]; outms: [NCORES,128,PM]; hbds: [NCORES,128,T,PS]."""
    x2 = np.einsum("pd,pd->p", x, x, dtype=np.float32)
    # cols [0, PA): h accumulated as y2 - shift - 2G, min'd on-chip
    ma = outas.astype(np.float32).min(axis=(0, 1)) + Y2_SHIFT
    # cols [PA, PA+PS): raw h tiles, min on host
    mb = hbds.astype(np.float32).min(axis=(0, 1, 2)) + Y2_SHIFT
    # cols [PA+PS, P): acc of max(G - y2/2); sq = x2 - 2*max
    mm = np.float32(-2.0) * outms.astype(np.float32).max(axis=(0, 1))
    m = np.concatenate([ma, mb, mm])
    sq_min = np.float32((x2 + m).min())
    return np.sqrt(np.maximum(sq_min, np.float32(0.0)), dtype=np.float32)


def kernel(
    predicted_transaction_company: np.ndarray,
    future_transaction_companies_inc_current_data: np.ndarray,
) -> np.ndarray:
    x = np.asarray(predicted_transaction_company, dtype=np.float32)[0]
    y = np.asarray(future_transaction_companies_inc_current_data, dtype=np.float32)[0]

    nc = _get_module()
    in_maps = _prepare_inputs(x, y)
    res = bass_utils.run_bass_kernel_spmd(nc, in_maps, core_ids=list(range(NCORES)))
    outas = np.stack([r["outa"] for r in res.results])
    outms = np.stack([r["outm"] for r in res.results])
    hbds = np.stack([r["hbd"] for r in res.results])
    return _postprocess(x, outas, outms, hbds)


# revision 27
# speedup vs baseline: 1.0225x; 1.0099x over previous
"""Min-Euclidean-distance retrieval kernel for Trainium2 (8 NeuronCores).

Reference computation:
    x: [1, 2048, 512], y: [1, 65536, 512] (fp32)
    sq[p, r] = ||x_p||^2 + ||y_r||^2 - 2 <x_p, y_r>
    out = min over (p, r) of sqrt(max(sq, 0))

Sharding: candidate pool (R) split across 8 cores, 8192 candidates each.
Host pre-arranges both GEMM operands partition-major in fp8 so each DMA
moves contiguous per-partition runs and the contraction dim lands on SBUF
partitions with no on-chip transposes.

Per core the hot loop is 64 candidate tiles of [128 cand x 2048 queries],
8 fp8 DoubleRow matmuls each (216ns issue floor -> 1.728us/tile, the PE
roofline).  The 2048 query columns split three ways so no sidecar engine
exceeds the PE tile period:
  cols    0-1023  ScalarE ACTIVATE h=-2*pa+(y2-512) (with 1024:1536,
                  1.54us) then VectorE fp16 tensor_tensor running min
                  into acc_a (0.69us)
  cols 1024-1535  same ACTIVATE output, shipped raw to DRAM on the sync
                  HWDGE ring (host takes the min over tiles)
  cols 1536-2047  VectorE scalar_tensor_tensor straight off PSUM:
                  acc_m = max(acc_m, pb - y2/2)  (0.75us; bias + running
                  reduce fused, nothing shipped)
VectorE ~1.49us/tile, ScalarE ~1.59us/tile, both under the PE's 1.728us.

Queue discipline:
  - scalar (Activation) queue: head x/y DMA issues only BEFORE the first
    ACTIVATE; nothing mid-stream (ACTIVATE shares the queue).
  - gpsimd: memsets only (SWDGE transfers measured ~65 GB/s and its
    block-exit DRAIN blocked 11us when it carried y tiles).
  - h tiles: 16 single-tile buffers -- deep enough that ACTIVATE's WAR
    wait on the hbd ship of t-16 never binds, with no pair-packing WAW
    semaphores on the scalar queue.
  - x arrives as 8 consumption-ordered 128KB chunk DMAs alternating
    sync/scalar (first real MM ~8.7us); y tiles 0-1 lead on sync, the
    rest stream in 4-tile groups from inside the loop on sync,
    interleaved with the hbd ships.

Garbage warm-up matmuls (memset FIRST on gpsimd, before everything)
bridge the PE clock ramp across the input-DMA window.  The final tile's
ACTIVATE is split in two so the closing tensor_tensor min and the outa
ship start ~0.4us earlier.

The per-query ||x_p||^2 term commutes with the min over candidates and
is added on the host, with the final min across lanes/cores/tiles and
the (monotone) sqrt. fp8 GEMM + fp16 epilogue measure ~1.6e-3 relative
error on the final distance, well inside the 2e-2 tolerance.
"""

import os
import sys

# Recover automatically if a previous process left the NeuronCores wedged.
os.environ.setdefault("NEURON_RT_RESET_CORES", "1")

for _p in ("/opt/trn_rl_repo", "/root/.axon_site/_ro/trn_rl_repo"):
    if _p not in sys.path:
        sys.path.append(_p)

import ml_dtypes
import numpy as np

import concourse.bass as bass
import concourse.mybir as mybir
import concourse.tile as tile
from concourse import bacc, bass_utils

P = 2048          # queries
R = 65536         # candidates (full)
D = 512           # feature dim
NCORES = 8
R_LOC = R // NCORES      # 8192 candidates per core
P_CHUNKS = P // 512      # 4 query chunks (one PSUM bank each)
R_TILES = R_LOC // 128   # 64 candidate tiles
K_TILES = D // 128       # 4 contraction tiles (2 DoubleRow passes)
PA = 1024                # query cols on the ScalarE+VectorE-min path
PS = 512                 # query cols shipped raw to the host
PM = 512                 # query cols on the fused VectorE max path
PAS = PA + PS

F32 = mybir.dt.float32
MM_DT = mybir.dt.float8e4
MM_NP = ml_dtypes.float8_e4m3
ACC_DT, ACC_NP = mybir.dt.float16, np.float16
# The a-path epilogue runs in fp16. A constant shift keeps the values that
# matter (near the global min, sq ~ 650 => h ~ 150) small; fp16 quantum
# there is ~0.125, negligible next to the fp8 GEMM noise.
Y2_SHIFT = np.float32(512.0)
# Garbage matmuls bridging the input-DMA window: enough to keep the PE
# busy from ~8.2us until the first x chunk lands (~10.5us) -- a gap there
# resets the clock-ramp timer and the first ~10 real matmuls then run at
# the mid pstate (426ns instead of 216ns, ~1.8us lost).  Two extra beyond
# the measured bridge insure against slower DMA on a cold run; overshoot
# costs only ~213ns each.
N_WARM = 12


def _build_module() -> bass.Bass:
    nc = bacc.Bacc("TRN2", target_bir_lowering=False, debug=False)

    # Host-prepared layouts (partition-major, contiguous per partition):
    #   xt[q, c, k, j]   = x[c*512 + j, k*128 + q]
    #   yt[q, t, k, s]   = y[t*128 + s, k*128 + q]  (t-major: one candidate
    #                      tile = one contiguous 512B-per-partition slice)
    #   y2bh[lane, 0, t] = ||y_r||^2 - Y2_SHIFT for r = t*128 + lane
    #   y2bh[lane, 1, t] = ||y_r||^2 / 2
    xt = nc.dram_tensor("xt", [128, P_CHUNKS, K_TILES, 512], MM_DT,
                        kind="ExternalInput")
    yt = nc.dram_tensor("yt", [128, R_TILES, K_TILES, 128], MM_DT,
                        kind="ExternalInput")
    y2bh = nc.dram_tensor("y2bh", [128, 2, R_TILES], F32, kind="ExternalInput")
    # outa[lane, j<PA]: min over tiles t of (y2[t*128+lane] - 512 - 2 G[., j])
    outa = nc.dram_tensor("outa", [128, PA], ACC_DT, kind="ExternalOutput")
    # outm[lane, j<PM]: max over tiles t of (G[., PA+PS+j] - y2[t*128+lane]/2)
    outm = nc.dram_tensor("outm", [128, PM], ACC_DT, kind="ExternalOutput")
    # hbd[lane, t, j] = y2[t*128+lane] - 512 - 2 G[t*128+lane, PA+j] (no min)
    hbd = nc.dram_tensor("hbd", [128, R_TILES, PS], ACC_DT,
                         kind="ExternalOutput")

    with tile.TileContext(nc) as tc:
        with (
            tc.tile_pool(name="big", bufs=1) as big,
            tc.tile_pool(name="hpool", bufs=16) as hpool,
            tc.tile_pool(name="psa", bufs=2, space="PSUM") as psa,
            tc.tile_pool(name="psb", bufs=2, space="PSUM") as psb,
        ):
            garb = big.tile([128, 2, 512], MM_DT)
            xt_sb = big.tile([128, P_CHUNKS, K_TILES, 512], MM_DT)
            yt_sb = big.tile([128, R_TILES, K_TILES, 128], MM_DT)
            y2bh_sb = big.tile([128, 2, R_TILES], F32)
            acc_a = big.tile([128, PA], ACC_DT)
            acc_m = big.tile([128, PM], ACC_DT)

            # GpSimd zeroes the warm-up operand FIRST (so the warm-ups are
            # schedulable immediately), then seeds the accumulators.
            nc.gpsimd.memset(garb[:], 0)
            nc.gpsimd.memset(acc_a[:], float("inf"))
            nc.gpsimd.memset(acc_m[:], float("-inf"))

            # Leading-edge DMAs: y0-1 lead (the first LDWEIGHTS gate), then
            # the 8 x chunk-pairs in exact MM-consumption order alternating
            # scalar/sync, then the next y tiles.  All of scalar's issues
            # are pre-ACTIVATE.
            nc.sync.dma_start(yt_sb[:, 0:2], yt.ap()[:, 0:2])
            nc.scalar.dma_start(xt_sb[:, 0, 0:2], xt.ap()[:, 0, 0:2])
            nc.sync.dma_start(xt_sb[:, 1, 0:2], xt.ap()[:, 1, 0:2])
            nc.scalar.dma_start(xt_sb[:, 2, 0:2], xt.ap()[:, 2, 0:2])
            nc.sync.dma_start(xt_sb[:, 3, 0:2], xt.ap()[:, 3, 0:2])
            nc.scalar.dma_start(y2bh_sb[:], y2bh.ap())
            nc.scalar.dma_start(xt_sb[:, 0, 2:4], xt.ap()[:, 0, 2:4])
            nc.sync.dma_start(xt_sb[:, 1, 2:4], xt.ap()[:, 1, 2:4])
            nc.scalar.dma_start(xt_sb[:, 2, 2:4], xt.ap()[:, 2, 2:4])
            nc.sync.dma_start(xt_sb[:, 3, 2:4], xt.ap()[:, 3, 2:4])
            nc.sync.dma_start(yt_sb[:, 2:4], yt.ap()[:, 2:4])
            nc.scalar.dma_start(yt_sb[:, 4:8], yt.ap()[:, 4:8])

            # Warm-up matmuls on the zeroed garbage tile: the PE would
            # otherwise sit idle waiting for x/y and spend the first real
            # tiles at the ramp-up clock.
            pwarm = psb.tile([128, PM], F32, name="pb")
            for _ in range(N_WARM):
                nc.tensor.matmul(
                    pwarm[:, 0:256],
                    lhsT=garb[:, :, 0:128],
                    rhs=garb[:, :, 0:256],
                    start=True,
                    stop=True,
                    perf_mode=mybir.MatmulPerfMode.DoubleRow,
                )

            prev_h = None
            for t in range(R_TILES):
                if t % 4 == 2 and t < 58:
                    # y tiles for group g..g+3 (issued 6+ tiles ahead).
                    # Issued at t=2 mod 4 so the first issue's semaphore
                    # recycling lands AFTER the first fused-max on the
                    # vector queue -- at t=0 its guard blocked STT(0) on an
                    # unrelated head x-chunk transfer for ~2us.
                    g = 6 + t
                    nc.sync.dma_start(yt_sb[:, g : g + 4], yt.ap()[:, g : g + 4])
                pa = psa.tile([128, PAS], F32, name="pa")
                pb = psb.tile([128, PM], F32, name="pb")
                # kk outer keeps the stationary operand loaded across
                # chunks; c3 (the fused-max path) last so pa completes at
                # MM#7 and ScalarE starts one MM early.
                for kk in range(K_TILES // 2):
                    for c in range(P_CHUNKS):
                        dst = (pa[:, c * 512 : (c + 1) * 512]
                               if c < 3 else pb[:])
                        nc.tensor.matmul(
                            dst,
                            lhsT=yt_sb[:, t, 2 * kk : 2 * kk + 2],
                            rhs=xt_sb[:, c, 2 * kk : 2 * kk + 2],
                            start=(kk == 0),
                            stop=(kk == K_TILES // 2 - 1),
                            perf_mode=mybir.MatmulPerfMode.DoubleRow,
                        )
                h = hpool.tile([128, PAS], ACC_DT, name="h")
                if t == R_TILES - 1:
                    # Split so the closing min (and outa) starts early.
                    nc.scalar.activation(
                        out=h[:, 0:PA], in_=pa[:, 0:PA],
                        func=mybir.ActivationFunctionType.Identity,
                        bias=y2bh_sb[:, 0, t : t + 1], scale=-2.0,
                    )
                    nc.scalar.activation(
                        out=h[:, PA:PAS], in_=pa[:, PA:PAS],
                        func=mybir.ActivationFunctionType.Identity,
                        bias=y2bh_sb[:, 0, t : t + 1], scale=-2.0,
                    )
                else:
                    nc.scalar.activation(
                        out=h[:],
                        in_=pa[:],
                        func=mybir.ActivationFunctionType.Identity,
                        bias=y2bh_sb[:, 0, t : t + 1],
                        scale=-2.0,
                    )
                # The a-path min runs one tile late so the fused-max (which
                # frees the psb buffer) never queues behind it.
                if prev_h is not None:
                    nc.vector.tensor_tensor(
                        out=acc_a[:], in0=acc_a[:], in1=prev_h[:, 0:PA],
                        op=mybir.AluOpType.min,
                    )
                nc.vector.scalar_tensor_tensor(
                    out=acc_m[:], in0=pb[:], scalar=y2bh_sb[:, 1, t : t + 1],
                    in1=acc_m[:],
                    op0=mybir.AluOpType.subtract, op1=mybir.AluOpType.max,
                )
                if t == R_TILES - 1:
                    nc.sync.dma_start(outm.ap(), acc_m[:])
                nc.sync.dma_start(hbd.ap()[:, t], h[:, PA:PAS])
                prev_h = h
            nc.vector.tensor_tensor(
                out=acc_a[:], in0=acc_a[:], in1=prev_h[:, 0:PA],
                op=mybir.AluOpType.min,
            )
            nc.scalar.dma_start(outa.ap(), acc_a[:])
    nc.compile()
    return nc


_module_cache: bass.Bass | None = None


def _get_module() -> bass.Bass:
    global _module_cache
    if _module_cache is None:
        _module_cache = _build_module()
    return _module_cache


def _prepare_inputs(x: np.ndarray, y: np.ndarray):
    """Host-side sharding/layout prep. Returns per-core input maps."""
    # xt[q, c, k, j] = x[c*512 + j, k*128 + q]
    xt4 = x.T.reshape(K_TILES, 128, P_CHUNKS, 512)
    xt = np.ascontiguousarray(xt4.transpose(1, 2, 0, 3).astype(MM_NP))
    in_maps = []
    for cc in range(NCORES):
        yc = y[cc * R_LOC : (cc + 1) * R_LOC]
        # yt[q, t, k, s] = yc[t*128 + s, k*128 + q]
        a = yc.reshape(R_TILES, 128, K_TILES, 128)
        yct = np.ascontiguousarray(a.transpose(3, 0, 2, 1).astype(MM_NP))
        y2 = np.einsum("rd,rd->r", yc, yc, dtype=np.float32)
        y2bh = np.stack([
            (y2 - Y2_SHIFT).reshape(R_TILES, 128).T,
            (y2 * np.float32(0.5)).reshape(R_TILES, 128).T,
        ], axis=1)
        in_maps.append({"xt": xt, "yt": yct,
                        "y2bh": np.ascontiguousarray(y2bh)})
    return in_maps


def _postprocess(x: np.ndarray, outas: np.ndarray, outms: np.ndarray,
                 hbds: np.ndarray) -> np.ndarray:
    """outas: [NCORES,128,PA]; outms: [NCORES,128,PM]; hbds: [NCORES,128,T,PS]."""
    x2 = np.einsum("pd,pd->p", x, x, dtype=np.float32)
    # cols [0, PA): h accumulated as y2 - shift - 2G, min'd on-chip
    ma = outas.astype(np.float32).min(axis=(0, 1)) + Y2_SHIFT
    # cols [PA, PA+PS): raw h tiles, min on host
    mb = hbds.astype(np.float32).min(axis=(0, 1, 2)) + Y2_SHIFT
    # cols [PA+PS, P): acc of max(G - y2/2); sq = x2 - 2*max
    mm = np.float32(-2.0) * outms.astype(np.float32).max(axis=(0, 1))
    m = np.concatenate([ma, mb, mm])
    sq_min = np.float32((x2 + m).min())
    return np.sqrt(np.maximum(sq_min, np.float32(0.0)), dtype=np.float32)


def kernel(
    predicted_transaction_company: np.ndarray,
    future_transaction_companies_inc_current_data: np.ndarray,
) -> np.ndarray:
    x = np.asarray(predicted_transaction_company, dtype=np.float32)[0]
    y = np.asarray(future_transaction_companies_inc_current_data, dtype=np.float32)[0]

    nc = _get_module()
    in_maps = _prepare_inputs(x, y)
    res = bass_utils.run_bass_kernel_spmd(nc, in_maps, core_ids=list(range(NCORES)))
    outas = np.stack([r["outa"] for r in res.results])
    outms = np.stack([r["outm"] for r in res.results])
    hbds = np.stack([r["hbd"] for r in res.results])
    return _postprocess(x, outas, outms, hbds)
